# revision 2
# baseline (speedup 1.0000x reference)
"""Causal attention head (RoPE) kernel for 8 Trainium2 NeuronCores — v2.

Sharding: 8 cores = 2 batches x 4 head-groups (4 heads each), no
cross-device comms. v2 restructures attention around PE tile-position
concurrency (16x 32x32 subarrays):

  - scores: per key-block j, the two heads of a pair run as adjacent
    row-tiled matmuls (K=64 at rows 0-63 / 64-127) into ONE pair-shared
    PSUM tile [128 keys, w(h0) | w(h1)] -> both streams overlap on the
    PE, one exp instruction covers both heads.
  - PV: per key-block, 3 col-tiled rounds over all 4 heads:
    [h0 V | h1 V] (M=64 at cols 0/64), [h2 V | h3 V], [4x ones strips
    at cols 0/32/64/96] -> 3 streams instead of 4 serial M=65 streams,
    denominators land in po3 rows {0,32,64,96}.
  - loop is qc-major: chunk qc braids chunk qc-1's PV triples into the
    m0 score phase (triple(c,j') issued just before SQ(m0,qc,j') so the
    bf16 pt tile (bufs=1) is consumed before its next write); the m1
    phase carries projection/V-proj filler. ACT (exp) is the designed
    critical path; PE stalls are harmless as long as ACT never waits.
  - exp feeds from PSUM in-order; ACT runs continuously from ~t=12us.
"""

import os
import sys
from contextlib import ExitStack

import numpy as np

for _p in ("/opt/trn_rl_repo", "/root/.axon_site/_ro/trn_rl_repo"):
    if os.path.isdir(_p) and _p not in sys.path:
        sys.path.append(_p)

import ml_dtypes

import concourse.bass as bass
import concourse.mybir as mybir
import concourse.tile as tile
from concourse import bacc
from concourse.bass_utils import run_bass_kernel_spmd

P = 128
T = 2048
CIN = 1024
NHC = 4          # heads per core
HS = 64
DOUT = NHC * HS  # 256
NCT = CIN // P   # 8 contraction tiles
NCP = NCT // 2   # 4 fp8 DoubleRow contraction pairs
SCALE = 1.0 / 32.0  # 1024 ** -0.5

F32 = mybir.dt.float32
BF16 = mybir.dt.bfloat16
F8 = mybir.dt.float8e4
DR = mybir.MatmulPerfMode.DoubleRow


def _build_nc():
    nc = bacc.Bacc("TRN2")

    xbT = nc.dram_tensor("xbT", [CIN, T], BF16, kind="ExternalInput").ap()
    x8T = nc.dram_tensor("x8T", [CIN, T], F8, kind="ExternalInput").ap()
    wq8 = nc.dram_tensor("wq8", [P, NCT * DOUT], F8, kind="ExternalInput").ap()
    wk8 = nc.dram_tensor("wk8", [P, NCT * DOUT], F8, kind="ExternalInput").ap()
    wvT = nc.dram_tensor("wvT", [P, NCT * DOUT], BF16, kind="ExternalInput").ap()
    cos4 = nc.dram_tensor("cos4", [P, T], BF16, kind="ExternalInput").ap()
    sin4 = nc.dram_tensor("sin4", [P, T], BF16, kind="ExternalInput").ap()
    utri2 = nc.dram_tensor("utri2", [P, 2 * P], BF16, kind="ExternalInput").ap()
    outV = nc.dram_tensor("outV", [NHC * HS, T], F32, kind="ExternalOutput").ap()
    outD = nc.dram_tensor("outD", [NHC, T], F32, kind="ExternalOutput").ap()

    with tile.TileContext(nc) as tc, ExitStack() as ctx:
        const_pool = ctx.enter_context(tc.tile_pool(name="const", bufs=1))
        wpool = ctx.enter_context(tc.tile_pool(name="w", bufs=1))
        xpool = ctx.enter_context(tc.tile_pool(name="x", bufs=1))
        qkpool = ctx.enter_context(tc.tile_pool(name="qk", bufs=1))
        mpool = ctx.enter_context(tc.tile_pool(name="m", bufs=2))
        vpool = ctx.enter_context(tc.tile_pool(name="vaug", bufs=1))
        ptpool = ctx.enter_context(tc.tile_pool(name="pt", bufs=1))
        opool = ctx.enter_context(tc.tile_pool(name="ob", bufs=2))

        pp_acc = ctx.enter_context(tc.tile_pool(name="pp_acc", bufs=1, space="PSUM"))
        pp_s = ctx.enter_context(tc.tile_pool(name="pp_s", bufs=2, space="PSUM"))
        pp_po = ctx.enter_context(tc.tile_pool(name="pp_po", bufs=1, space="PSUM"))

        # ---- early input DMAs: only what the critical path (pair-0 half-0
        # projections -> first scores -> first exps) needs. Everything else
        # (wv, utri2, xb, x8 halves 2/3) is emitted AFTER the attention
        # braid so its scheduler priority can never preempt the exp feed.
        # weights arrive pre-arranged [128, NCT*DOUT] from the host so the
        # DMA is a flat copy with 2KB per-partition lines (rearranged
        # loads had 256B lines and dominated the startup latency)
        w_tiles = {}
        w_q = wpool.tile([P, NCT * DOUT], F8, tag="wq", name="w_q")
        nc.sync.dma_start(w_q[:], wq8)
        xb_r = xbT.rearrange("(n p) t -> p n t", p=P)
        x8_r = x8T.rearrange("(n p) t -> p n t", p=P)
        # x8 per-cp full-T tiles, loaded as T-halves. The half-0 pieces
        # (all the first projections need) go on the two fast hwdge rings
        # (sync + scalar); the half-1 pieces ride the slower gpsimd swdge.
        xs8_t = [
            xpool.tile([P, 2 * T], F8, tag=f"x8_{cp}", name=f"x8_{cp}")
            for cp in range(NCP)
        ]

        def load_x8(cp, hf, eng):
            eng.dma_start(
                xs8_t[cp].rearrange("p (n t) -> p n t", n=2)
                [:, :, hf * 1024:(hf + 1) * 1024],
                x8_r[:, 2 * cp:2 * cp + 2, hf * 1024:(hf + 1) * 1024],
            )

        # early critical loads, ordered by first-use time per ring
        w_k = wpool.tile([P, NCT * DOUT], F8, tag="wk", name="w_k")
        cos_s = const_pool.tile([P, T], BF16, tag="cos")
        sin_s = const_pool.tile([P, T], BF16, tag="sin")
        load_x8(0, 0, nc.scalar)
        load_x8(1, 0, nc.sync)
        load_x8(2, 0, nc.scalar)
        nc.sync.dma_start(w_k[:], wk8)
        load_x8(3, 0, nc.sync)
        nc.scalar.dma_start(cos_s[:, 0:1024], cos4[:, 0:1024])
        nc.sync.dma_start(sin_s[:, 0:1024], sin4[:, 0:1024])
        nc.scalar.dma_start(cos_s[:, 1024:T], cos4[:, 1024:T])
        nc.sync.dma_start(sin_s[:, 1024:T], sin4[:, 1024:T])
        for cp in range(NCP):
            load_x8(cp, 1, nc.gpsimd)
        utri_s = const_pool.tile([P, 2 * P], BF16, tag="utri")
        w_v = wpool.tile([P, NCT * DOUT], BF16, tag="wv", name="w_v")
        w_tiles.update(q=w_q, k=w_k, v=w_v)

        # xb: per-cp half-T tiles (2KB lines); half 0 feeds vproj 0-3
        xsb_t = [[None] * NCP for _ in range(2)]

        def load_xb(hf):
            for cp in range(NCP):
                xt = xpool.tile([P, 2 * 1024], BF16, tag=f"xb{hf}_{cp}",
                                name=f"xb{hf}_{cp}")
                nc.gpsimd.dma_start(
                    xt.rearrange("p (n t) -> p n t", n=2),
                    xb_r[:, 2 * cp:2 * cp + 2, hf * 1024:(hf + 1) * 1024],
                )
                xsb_t[hf][cp] = xt

        # roped q/k as per-512-col quarter tiles (pair m: rows 0-63/64-127)
        qth = [[qkpool.tile([P, 512], BF16, tag=f"qt{m}_{h2}", name=f"qt{m}_{h2}")
                for h2 in range(4)] for m in range(2)]
        kth = [[qkpool.tile([P, 512], BF16, tag=f"kt{m}_{h2}", name=f"kt{m}_{h2}")
                for h2 in range(4)] for m in range(2)]
        va = [
            vpool.tile([P, NHC * (HS + 1)], BF16, tag=f"vaug{tb}", name=f"vaug{tb}")
            for tb in range(T // P)
        ]

        # projection/V-proj accumulators rotate across the acc bank and the
        # (otherwise idle between braids) po banks so the PE<->DVE drain
        # ping-pong double-buffers instead of serializing
        acc_rr = [(pp_acc, "acc"), (pp_po, "po1"), (pp_po, "po2"),
                  (pp_po, "po3")]
        acc_i = [0]

        def acc_tile(name):
            pool, tag = acc_rr[acc_i[0] % 4]
            acc_i[0] += 1
            return pool.tile([P, 512], F32, tag=tag, name=name)

        def proj_rope_h(m, wname, dsts, half, eng):
            """fp8 DR projection of one 1024-col half of an m-tile + RoPE."""
            w_r = w_tiles[wname].rearrange("p (n d) -> p n d", n=NCT)
            ra = mpool.tile([P, 1024], BF16, tag="ra", name=f"ra{wname}{m}{half}")
            rp = mpool.tile([P, 1024], BF16, tag="rp", name=f"rp{wname}{m}{half}")
            for chh in range(2):
                qq = half * 2 + chh
                cs = slice(qq * 512, (qq + 1) * 512)
                hs = slice(chh * 512, (chh + 1) * 512)
                ps = acc_tile(f"pj{wname}{m}{qq}")
                for cp in range(NCP):
                    x8pr = xs8_t[cp].rearrange("p (n t) -> p n t", n=2)
                    nc.tensor.matmul(
                        ps[:],
                        lhsT=w_r[:, 2 * cp:2 * cp + 2, m * P:(m + 1) * P],
                        rhs=x8pr[:, :, qq * 512:(qq + 1) * 512],
                        perf_mode=DR,
                        start=(cp == 0),
                        stop=(cp == NCP - 1),
                    )
                nc.vector.tensor_mul(ra[:, hs], ps[:], cos_s[:, cs])
                nc.vector.tensor_mul(rp[:, hs], ps[:], sin_s[:, cs])
            sw = mpool.tile([P, 1024], BF16, tag="rs", name=f"rs{wname}{m}{half}")
            for blk in range(4):
                s0 = (blk ^ 1) * 32
                eng.dma_start(sw[blk * 32:(blk + 1) * 32, :], rp[s0:s0 + 32, :])
            for chh in range(2):
                hs = slice(chh * 512, (chh + 1) * 512)
                nc.vector.tensor_sub(dsts[half * 2 + chh][:], ra[:, hs], sw[:, hs])

        def vproj(tbp):
            """bf16 V proj of t-blocks (2*tbp, 2*tbp+1) into natural layout.

            Must only be emitted OUTSIDE open PV chunks — the rotating
            accumulator may land on a po tag, and a WAR against a live po
            accumulator would deadlock through the chunk's own triples."""
            pv = acc_tile(f"pv{tbp}")
            wv_r = w_tiles["v"].rearrange("p (n d) -> p n d", n=NCT)
            for i in range(2):
                tb = 2 * tbp + i
                tb8 = tb % 8
                for c in range(NCT):
                    xb_c = xsb_t[tb // 8][c // 2].rearrange(
                        "p (n t) -> p n t", n=2)
                    nc.tensor.matmul(
                        pv[:, i * DOUT:(i + 1) * DOUT],
                        lhsT=xb_c[:, c % 2, tb8 * P:(tb8 + 1) * P],
                        rhs=wv_r[:, c, :],
                        start=(c == 0),
                        stop=(c == NCT - 1),
                        skip_group_check=True,
                    )
            pv_r = pv.rearrange("p (i h d) -> p i h d", i=2, h=NHC)
            for i in range(2):
                vt_r = va[2 * tbp + i].rearrange("p (h e) -> p h e", e=HS + 1)
                nc.gpsimd.memset(vt_r[:, :, HS:HS + 1], 1.0)
                nc.vector.tensor_copy(vt_r[:, :, 0:HS], pv_r[:, i, :, :])

        # ---- attention machinery ----
        pts = {}        # (m, j) -> (pt tile, w)
        po_cur = {}     # name -> live PSUM accum tiles for current PV chunk

        def blk_w(qc, j):
            col0 = max(0, j * P - qc * 512)
            return 512 - col0, col0

        def sq(m, qc, j):
            """Pair-shared scores for key block j of chunk qc + exp + mask.

            Tile layout [128 keys, 1024]: h0 at cols [0:w] (bank 0), h1 at
            [512:512+w] (bank 1) — the two heads' concurrently-streaming
            matmuls must never write the same PSUM bank (hw fault)."""
            w, col0 = blk_w(qc, j)
            ps = pp_s.tile([P, 1024], F32, tag="ps", name=f"ps{m}_{qc}_{j}")
            pt = ptpool.tile([P, 1024], BF16, tag=f"pt{m}_{j}",
                             name=f"pt{m}_{qc}_{j}")
            pts[(m, j)] = (pt, w)
            kq, kk0 = (j * P) // 512, (j * P) % 512
            qt = qth[m][qc]
            for hi in range(2):
                r0 = hi * HS
                nc.tensor.matmul(
                    ps[:, hi * 512:hi * 512 + w],
                    lhsT=kth[m][kq][r0:r0 + HS, kk0:kk0 + P],
                    rhs=qt[r0:r0 + HS, col0:col0 + w],
                    start=True,
                    stop=True,
                    tile_position=(r0, 0),
                )
            ps3 = ps.rearrange("p (n c) -> p n c", n=2)[:, :, 0:w]
            pt3 = pt.rearrange("p (n c) -> p n c", n=2)[:, :, 0:w]
            nc.scalar.activation(
                pt3, ps3, mybir.ActivationFunctionType.Exp, scale=SCALE,
            )
            if col0 > 0 or j * P == qc * 512:  # diagonal block: causal mask
                ptm = pt.rearrange("p (n c) -> p n c", n=2)[:, :, 0:P]
                ut3 = utri_s.rearrange("p (n c) -> p n c", n=2)
                nc.vector.tensor_mul(ptm, ptm, ut3)

        def pv_open(c):
            po1 = pp_po.tile([P, 512], F32, tag="po1", name=f"po1_{c}")
            po2 = pp_po.tile([P, 512], F32, tag="po2", name=f"po2_{c}")
            po3 = pp_po.tile([P, 512], F32, tag="po3", name=f"po3_{c}")
            po_cur.update(po1=po1, po2=po2, po3=po3)

        def pv_triple(c, j, last):
            """3 col-tiled PV rounds for key block j of chunk c (4 heads)."""
            w, col0 = blk_w(c, j)
            st = (j == 0)
            po1, po2, po3 = po_cur["po1"], po_cur["po2"], po_cur["po3"]
            for m, po in ((0, po1), (1, po2)):
                pt, _w = pts[(m, j)]
                for hi in range(2):
                    h = 2 * m + hi
                    nc.tensor.matmul(
                        po[hi * HS:hi * HS + HS, col0:512],
                        lhsT=va[j][:, h * (HS + 1):h * (HS + 1) + HS],
                        rhs=pt[:, hi * 512:hi * 512 + w],
                        start=st,
                        stop=last,
                        skip_group_check=True,
                        tile_position=(0, hi * HS),
                    )
            for m in (0, 1):
                pt, _w = pts[(m, j)]
                for hi in range(2):
                    h = 2 * m + hi
                    nc.tensor.matmul(
                        po3[32 * h:32 * h + 1, col0:512],
                        lhsT=va[j][:, h * (HS + 1) + HS:h * (HS + 1) + HS + 1],
                        rhs=pt[:, hi * 512:hi * 512 + w],
                        start=st,
                        stop=last,
                        skip_group_check=True,
                        tile_position=(0, 32 * h),
                    )

        def pv_drain(c):
            q0 = c * 512
            po1, po2, po3 = po_cur["po1"], po_cur["po2"], po_cur["po3"]
            ob1 = opool.tile([P, 512], F32, tag="ob1", name=f"ob1_{c}")
            nc.vector.tensor_copy(ob1[:], po1[:])
            nc.sync.dma_start(outV[0:P, q0:q0 + 512], ob1[:])
            ob2 = opool.tile([P, 512], F32, tag="ob2", name=f"ob2_{c}")
            nc.vector.tensor_copy(ob2[:], po2[:])
            nc.sync.dma_start(outV[P:2 * P, q0:q0 + 512], ob2[:])
            obd = opool.tile([P, 512], F32, tag="obd", name=f"obd_{c}")
            nc.vector.tensor_copy(obd[:], po3[:])
            nc.sync.dma_start(
                outD[:, q0:q0 + 512],
                obd.rearrange("(n r) c -> n r c", r=32)[:, 0, :],
            )

        # ---- emission schedule. Program order must respect dataflow (the
        # dep tracker is a linear scan), so producers are interleaved at
        # their proper spots; the score/exp/PV braid is priority-boosted
        # so the scheduler never lets filler (projections, V-projs, late
        # loads) preempt the exp feed — filler runs in PE/DVE idle slots.
        # Emission = intended execution order; projections and V-projs are
        # interleaved into specific m0 phases sized so each phase's PE
        # work fits under that phase's ACT (exp) budget.
        nc.sync.dma_start(utri_s[:], utri2)
        proj_rope_h(0, "q", qth[0], 0, nc.sync)
        proj_rope_h(0, "k", kth[0], 0, nc.sync)
        for j in range(4):
            sq(0, 0, j)
        proj_rope_h(1, "q", qth[1], 0, nc.gpsimd)
        proj_rope_h(1, "k", kth[1], 0, nc.gpsimd)
        for j in range(4):
            sq(1, 0, j)
        load_xb(0)
        load_xb(1)
        nc.sync.dma_start(w_v[:], wvT)
        vproj(0)
        vproj(1)
        pv_open(0)
        for j in range(4):
            pv_triple(0, j, j == 3)
        pv_drain(0)
        proj_rope_h(0, "q", qth[0], 1, nc.sync)
        proj_rope_h(0, "k", kth[0], 1, nc.sync)

        # per-chunk extra PE work for the m0 phase, placed one phase
        # before its consumers
        m0_extra = {
            1: [lambda: vproj(2), lambda: vproj(3)],
            2: [lambda: vproj(4), lambda: vproj(5),
                lambda: proj_rope_h(1, "q", qth[1], 1, nc.gpsimd),
                lambda: proj_rope_h(1, "k", kth[1], 1, nc.gpsimd)],
            3: [lambda: vproj(6), lambda: vproj(7)],
        }

        for qc in range(1, 4):
            nb = 4 * qc + 4
            extras = m0_extra[qc]
            for j in range(nb):
                sq(0, qc, j)
                if j % 2 == 1 and extras:
                    extras.pop(0)()
            while extras:
                extras.pop(0)()
            pv_open(qc)
            for j in range(nb):
                sq(1, qc, j)
                if j >= 2:
                    pv_triple(qc, j - 2, False)
            pv_triple(qc, nb - 2, False)
            pv_triple(qc, nb - 1, True)
            pv_drain(qc)

    nc.compile()
    return nc


_CACHE = {}


def _get_nc():
    if "nc" not in _CACHE:
        _CACHE["nc"] = _build_nc()
    return _CACHE["nc"]


def _host_inputs(x, Wq, Wk, Wv):
    bf = ml_dtypes.bfloat16
    f8 = ml_dtypes.float8_e4m3
    # RoPE tables (match reference: theta over hs/2 freqs with dim=n_emb)
    i = np.arange(HS // 2, dtype=np.float32)
    theta = np.float32(10000.0) ** (-2.0 * i / np.float32(CIN))
    pos = np.arange(T, dtype=np.float32)
    ang = pos[:, None] * theta[None, :]
    cosT = np.cos(ang).T.astype(np.float32)  # [32, T]
    sinT = np.sin(ang).T.astype(np.float32)
    cos4 = np.ascontiguousarray(np.tile(cosT, (4, 1))).astype(bf)
    sin4 = np.ascontiguousarray(
        np.tile(np.concatenate([-sinT, sinT], axis=0), (2, 1))
    ).astype(bf)  # rows: [-sin, +sin] x2
    utri_np = np.triu(np.ones((P, P), np.float32))
    utri2_np = np.ascontiguousarray(np.tile(utri_np, (1, 2))).astype(bf)

    perm = np.concatenate([np.arange(0, HS, 2), np.arange(1, HS, 2)])

    def warr(wT):
        # [1024, 256] -> [128, 8*256] so the device DMA is a flat copy
        return np.ascontiguousarray(
            wT.reshape(NCT, P, DOUT).transpose(1, 0, 2).reshape(P, NCT * DOUT)
        )

    in_maps = []
    for core in range(8):
        b, g = core // 4, core % 4
        idx = np.concatenate([(4 * g + h) * HS + perm for h in range(NHC)])
        xT = np.ascontiguousarray(x[b].T)
        m = {
            "xbT": xT.astype(bf),
            "x8T": xT.astype(f8),
            "wq8": warr(Wq[idx].T).astype(f8),
            "wk8": warr(Wk[idx].T).astype(f8),
            "wvT": warr(Wv[g * DOUT:(g + 1) * DOUT].T).astype(bf),
            "cos4": cos4,
            "sin4": sin4,
            "utri2": utri2_np,
        }
        in_maps.append(m)
    return in_maps


def kernel(x, Wq, Wk, Wv, _trace=False, _trace_kwargs=None):
    x = np.asarray(x)
    Wq, Wk, Wv = np.asarray(Wq), np.asarray(Wk), np.asarray(Wv)
    B = x.shape[0]
    nc = _get_nc()
    in_maps = _host_inputs(x, Wq, Wk, Wv)
    res = run_bass_kernel_spmd(
        nc, in_maps, list(range(8)), trace=_trace, **(_trace_kwargs or {})
    )
    out = np.zeros((B, T, CIN), np.float32)
    for core in range(8):
        b, g = core // 4, core % 4
        v = res.results[core]["outV"].reshape(NHC, HS, T)
        d = res.results[core]["outD"].reshape(NHC, 1, T)
        o = v / d
        out[b, :, g * DOUT:(g + 1) * DOUT] = o.reshape(DOUT, T).T
    if _trace:
        return out, res
    return out


# revision 3
# speedup vs baseline: 1.0344x; 1.0344x over previous
"""Causal attention head (RoPE) kernel for 8 Trainium2 NeuronCores — v2.

Sharding: 8 cores = 2 batches x 4 head-groups (4 heads each), no
cross-device comms. v2 restructures attention around PE tile-position
concurrency (16x 32x32 subarrays):

  - scores: per key-block j, the two heads of a pair run as adjacent
    row-tiled matmuls (K=64 at rows 0-63 / 64-127) into ONE pair-shared
    PSUM tile [128 keys, w(h0) | w(h1)] -> both streams overlap on the
    PE, one exp instruction covers both heads.
  - PV: per key-block, 3 col-tiled rounds over all 4 heads:
    [h0 V | h1 V] (M=64 at cols 0/64), [h2 V | h3 V], [4x ones strips
    at cols 0/32/64/96] -> 3 streams instead of 4 serial M=65 streams,
    denominators land in po3 rows {0,32,64,96}.
  - loop is qc-major: chunk qc braids chunk qc-1's PV triples into the
    m0 score phase (triple(c,j') issued just before SQ(m0,qc,j') so the
    bf16 pt tile (bufs=1) is consumed before its next write); the m1
    phase carries projection/V-proj filler. ACT (exp) is the designed
    critical path; PE stalls are harmless as long as ACT never waits.
  - exp feeds from PSUM in-order; ACT runs continuously from ~t=12us.
"""

import os
import sys
from contextlib import ExitStack

import numpy as np

for _p in ("/opt/trn_rl_repo", "/root/.axon_site/_ro/trn_rl_repo"):
    if os.path.isdir(_p) and _p not in sys.path:
        sys.path.append(_p)

import ml_dtypes

import concourse.bass as bass
import concourse.mybir as mybir
import concourse.tile as tile
from concourse import bacc
from concourse.bass_utils import run_bass_kernel_spmd

P = 128
T = 2048
CIN = 1024
NHC = 4          # heads per core
HS = 64
DOUT = NHC * HS  # 256
NCT = CIN // P   # 8 contraction tiles
NCP = NCT // 2   # 4 fp8 DoubleRow contraction pairs
SCALE = 1.0 / 32.0  # 1024 ** -0.5

F32 = mybir.dt.float32
BF16 = mybir.dt.bfloat16
F8 = mybir.dt.float8e4
DR = mybir.MatmulPerfMode.DoubleRow


def _build_nc():
    nc = bacc.Bacc("TRN2")

    xbT = nc.dram_tensor("xbT", [CIN, T], BF16, kind="ExternalInput").ap()
    x8T = nc.dram_tensor("x8T", [CIN, T], F8, kind="ExternalInput").ap()
    wq8 = nc.dram_tensor("wq8", [P, NCT * DOUT], F8, kind="ExternalInput").ap()
    wk8 = nc.dram_tensor("wk8", [P, NCT * DOUT], F8, kind="ExternalInput").ap()
    wvT = nc.dram_tensor("wvT", [P, NCT * DOUT], BF16, kind="ExternalInput").ap()
    cos4 = nc.dram_tensor("cos4", [P, T], BF16, kind="ExternalInput").ap()
    sin4 = nc.dram_tensor("sin4", [P, T], BF16, kind="ExternalInput").ap()
    utri2 = nc.dram_tensor("utri2", [P, 2 * P], BF16, kind="ExternalInput").ap()
    outV = nc.dram_tensor("outV", [NHC * HS, T], F32, kind="ExternalOutput").ap()
    outD = nc.dram_tensor("outD", [NHC, T], F32, kind="ExternalOutput").ap()

    with tile.TileContext(nc) as tc, ExitStack() as ctx:
        const_pool = ctx.enter_context(tc.tile_pool(name="const", bufs=1))
        wpool = ctx.enter_context(tc.tile_pool(name="w", bufs=1))
        xpool = ctx.enter_context(tc.tile_pool(name="x", bufs=1))
        qkpool = ctx.enter_context(tc.tile_pool(name="qk", bufs=1))
        mpool = ctx.enter_context(tc.tile_pool(name="m", bufs=2))
        vpool = ctx.enter_context(tc.tile_pool(name="vaug", bufs=1))
        ptpool = ctx.enter_context(tc.tile_pool(name="pt", bufs=1))
        opool = ctx.enter_context(tc.tile_pool(name="ob", bufs=2))

        pp_acc = ctx.enter_context(tc.tile_pool(name="pp_acc", bufs=1, space="PSUM"))
        pp_s = ctx.enter_context(tc.tile_pool(name="pp_s", bufs=2, space="PSUM"))
        pp_po = ctx.enter_context(tc.tile_pool(name="pp_po", bufs=1, space="PSUM"))

        # ---- early input DMAs: only what the critical path (pair-0 half-0
        # projections -> first scores -> first exps) needs. Everything else
        # (wv, utri2, xb, x8 halves 2/3) is emitted AFTER the attention
        # braid so its scheduler priority can never preempt the exp feed.
        # weights arrive pre-arranged [128, NCT*DOUT] from the host so the
        # DMA is a flat copy with 2KB per-partition lines (rearranged
        # loads had 256B lines and dominated the startup latency)
        w_tiles = {}
        w_q = wpool.tile([P, NCT * DOUT], F8, tag="wq", name="w_q")
        nc.sync.dma_start(w_q[:], wq8)
        xb_r = xbT.rearrange("(n p) t -> p n t", p=P)
        x8_r = x8T.rearrange("(n p) t -> p n t", p=P)
        # x8 per-cp full-T tiles, loaded as T-halves. The half-0 pieces
        # (all the first projections need) go on the two fast hwdge rings
        # (sync + scalar); the half-1 pieces ride the slower gpsimd swdge.
        xs8_t = [
            xpool.tile([P, 2 * T], F8, tag=f"x8_{cp}", name=f"x8_{cp}")
            for cp in range(NCP)
        ]

        def load_x8(cp, hf, eng):
            eng.dma_start(
                xs8_t[cp].rearrange("p (n t) -> p n t", n=2)
                [:, :, hf * 1024:(hf + 1) * 1024],
                x8_r[:, 2 * cp:2 * cp + 2, hf * 1024:(hf + 1) * 1024],
            )

        # early critical loads, ordered by first-use time per ring
        w_k = wpool.tile([P, NCT * DOUT], F8, tag="wk", name="w_k")
        cos_s = const_pool.tile([P, T], BF16, tag="cos")
        sin_s = const_pool.tile([P, T], BF16, tag="sin")
        load_x8(0, 0, nc.scalar)
        load_x8(1, 0, nc.sync)
        load_x8(2, 0, nc.scalar)
        nc.sync.dma_start(w_k[:], wk8)
        load_x8(3, 0, nc.sync)
        nc.scalar.dma_start(cos_s[:, 0:1024], cos4[:, 0:1024])
        nc.sync.dma_start(sin_s[:, 0:1024], sin4[:, 0:1024])
        nc.scalar.dma_start(cos_s[:, 1024:T], cos4[:, 1024:T])
        nc.sync.dma_start(sin_s[:, 1024:T], sin4[:, 1024:T])
        for cp in range(NCP):
            load_x8(cp, 1, nc.gpsimd)
        utri_s = const_pool.tile([P, 2 * P], BF16, tag="utri")
        w_v = wpool.tile([P, NCT * DOUT], BF16, tag="wv", name="w_v")
        w_tiles.update(q=w_q, k=w_k, v=w_v)

        # xb: per-cp half-T tiles (2KB lines); half 0 feeds vproj 0-3
        xsb_t = [[None] * NCP for _ in range(2)]

        def load_xb(hf):
            for cp in range(NCP):
                xt = xpool.tile([P, 2 * 1024], BF16, tag=f"xb{hf}_{cp}",
                                name=f"xb{hf}_{cp}")
                nc.gpsimd.dma_start(
                    xt.rearrange("p (n t) -> p n t", n=2),
                    xb_r[:, 2 * cp:2 * cp + 2, hf * 1024:(hf + 1) * 1024],
                )
                xsb_t[hf][cp] = xt

        # roped q/k as per-512-col quarter tiles (pair m: rows 0-63/64-127)
        qth = [[qkpool.tile([P, 512], BF16, tag=f"qt{m}_{h2}", name=f"qt{m}_{h2}")
                for h2 in range(4)] for m in range(2)]
        kth = [[qkpool.tile([P, 512], BF16, tag=f"kt{m}_{h2}", name=f"kt{m}_{h2}")
                for h2 in range(4)] for m in range(2)]
        va = [
            vpool.tile([P, NHC * (HS + 1)], BF16, tag=f"vaug{tb}", name=f"vaug{tb}")
            for tb in range(T // P)
        ]

        # projection/V-proj accumulators rotate across the acc bank and the
        # (otherwise idle between braids) po banks so the PE<->DVE drain
        # ping-pong double-buffers instead of serializing
        acc_rr = [(pp_acc, "acc"), (pp_po, "po1"), (pp_po, "po2"),
                  (pp_po, "po3")]
        acc_i = [0]

        def acc_tile(name):
            pool, tag = acc_rr[acc_i[0] % 4]
            acc_i[0] += 1
            return pool.tile([P, 512], F32, tag=tag, name=name)

        def proj_rope_h(m, wname, dsts, half, eng):
            """fp8 DR projection of one 1024-col half of an m-tile + RoPE."""
            w_r = w_tiles[wname].rearrange("p (n d) -> p n d", n=NCT)
            ra = mpool.tile([P, 1024], BF16, tag="ra", name=f"ra{wname}{m}{half}")
            rp = mpool.tile([P, 1024], BF16, tag="rp", name=f"rp{wname}{m}{half}")
            for chh in range(2):
                qq = half * 2 + chh
                cs = slice(qq * 512, (qq + 1) * 512)
                hs = slice(chh * 512, (chh + 1) * 512)
                ps = acc_tile(f"pj{wname}{m}{qq}")
                for cp in range(NCP):
                    x8pr = xs8_t[cp].rearrange("p (n t) -> p n t", n=2)
                    nc.tensor.matmul(
                        ps[:],
                        lhsT=w_r[:, 2 * cp:2 * cp + 2, m * P:(m + 1) * P],
                        rhs=x8pr[:, :, qq * 512:(qq + 1) * 512],
                        perf_mode=DR,
                        start=(cp == 0),
                        stop=(cp == NCP - 1),
                    )
                nc.vector.tensor_mul(ra[:, hs], ps[:], cos_s[:, cs])
                nc.vector.tensor_mul(rp[:, hs], ps[:], sin_s[:, cs])
            sw = mpool.tile([P, 1024], BF16, tag="rs", name=f"rs{wname}{m}{half}")
            for blk in range(4):
                s0 = (blk ^ 1) * 32
                eng.dma_start(sw[blk * 32:(blk + 1) * 32, :], rp[s0:s0 + 32, :])
            for chh in range(2):
                hs = slice(chh * 512, (chh + 1) * 512)
                nc.vector.tensor_sub(dsts[half * 2 + chh][:], ra[:, hs], sw[:, hs])

        def vproj(tbp):
            """bf16 V proj of t-blocks (2*tbp, 2*tbp+1) into natural layout.

            Must only be emitted OUTSIDE open PV chunks — the rotating
            accumulator may land on a po tag, and a WAR against a live po
            accumulator would deadlock through the chunk's own triples."""
            pv = acc_tile(f"pv{tbp}")
            wv_r = w_tiles["v"].rearrange("p (n d) -> p n d", n=NCT)
            for i in range(2):
                tb = 2 * tbp + i
                tb8 = tb % 8
                for c in range(NCT):
                    xb_c = xsb_t[tb // 8][c // 2].rearrange(
                        "p (n t) -> p n t", n=2)
                    nc.tensor.matmul(
                        pv[:, i * DOUT:(i + 1) * DOUT],
                        lhsT=xb_c[:, c % 2, tb8 * P:(tb8 + 1) * P],
                        rhs=wv_r[:, c, :],
                        start=(c == 0),
                        stop=(c == NCT - 1),
                        skip_group_check=True,
                    )
            pv_r = pv.rearrange("p (i h d) -> p i h d", i=2, h=NHC)
            for i in range(2):
                vt_r = va[2 * tbp + i].rearrange("p (h e) -> p h e", e=HS + 1)
                nc.gpsimd.memset(vt_r[:, :, HS:HS + 1], 1.0)
                nc.vector.tensor_copy(vt_r[:, :, 0:HS], pv_r[:, i, :, :])

        # ---- attention machinery ----
        pts = {}        # (m, j) -> (pt tile, w)
        po_cur = {}     # name -> live PSUM accum tiles for current PV chunk

        def blk_w(qc, j):
            col0 = max(0, j * P - qc * 512)
            return 512 - col0, col0

        def sq(m, qc, j):
            """Pair-shared scores for key block j of chunk qc + exp + mask.

            Tile layout [128 keys, 1024]: h0 at cols [0:w] (bank 0), h1 at
            [512:512+w] (bank 1) — the two heads' concurrently-streaming
            matmuls must never write the same PSUM bank (hw fault)."""
            w, col0 = blk_w(qc, j)
            ps = pp_s.tile([P, 1024], F32, tag="ps", name=f"ps{m}_{qc}_{j}")
            # m0 runs one chunk ahead of its PV consumer in the skewed
            # schedule, so its reused tags need double buffering
            nbufs = 2 if (m == 0 and j < 12) else 1
            pt = ptpool.tile([P, 1024], BF16, tag=f"pt{m}_{j}", bufs=nbufs,
                             name=f"pt{m}_{qc}_{j}")
            pts[(m, qc, j)] = (pt, w)
            kq, kk0 = (j * P) // 512, (j * P) % 512
            qt = qth[m][qc]
            for hi in range(2):
                r0 = hi * HS
                nc.tensor.matmul(
                    ps[:, hi * 512:hi * 512 + w],
                    lhsT=kth[m][kq][r0:r0 + HS, kk0:kk0 + P],
                    rhs=qt[r0:r0 + HS, col0:col0 + w],
                    start=True,
                    stop=True,
                    tile_position=(r0, 0),
                )
            ps3 = ps.rearrange("p (n c) -> p n c", n=2)[:, :, 0:w]
            pt3 = pt.rearrange("p (n c) -> p n c", n=2)[:, :, 0:w]
            nc.scalar.activation(
                pt3, ps3, mybir.ActivationFunctionType.Exp, scale=SCALE,
            )
            if col0 > 0 or j * P == qc * 512:  # diagonal block: causal mask
                ptm = pt.rearrange("p (n c) -> p n c", n=2)[:, :, 0:P]
                ut3 = utri_s.rearrange("p (n c) -> p n c", n=2)
                nc.vector.tensor_mul(ptm, ptm, ut3)

        def pv_open(c):
            po1 = pp_po.tile([P, 512], F32, tag="po1", name=f"po1_{c}")
            po2 = pp_po.tile([P, 512], F32, tag="po2", name=f"po2_{c}")
            po3 = pp_po.tile([P, 512], F32, tag="po3", name=f"po3_{c}")
            po_cur.update(po1=po1, po2=po2, po3=po3)

        def pv_triple(c, j, last):
            """3 col-tiled PV rounds for key block j of chunk c (4 heads)."""
            w, col0 = blk_w(c, j)
            st = (j == 0)
            po1, po2, po3 = po_cur["po1"], po_cur["po2"], po_cur["po3"]
            for m, po in ((0, po1), (1, po2)):
                pt, _w = pts[(m, c, j)]
                for hi in range(2):
                    h = 2 * m + hi
                    nc.tensor.matmul(
                        po[hi * HS:hi * HS + HS, col0:512],
                        lhsT=va[j][:, h * (HS + 1):h * (HS + 1) + HS],
                        rhs=pt[:, hi * 512:hi * 512 + w],
                        start=st,
                        stop=last,
                        skip_group_check=True,
                        tile_position=(0, hi * HS),
                    )
            for m in (0, 1):
                pt, _w = pts[(m, c, j)]
                for hi in range(2):
                    h = 2 * m + hi
                    nc.tensor.matmul(
                        po3[32 * h:32 * h + 1, col0:512],
                        lhsT=va[j][:, h * (HS + 1) + HS:h * (HS + 1) + HS + 1],
                        rhs=pt[:, hi * 512:hi * 512 + w],
                        start=st,
                        stop=last,
                        skip_group_check=True,
                        tile_position=(0, 32 * h),
                    )

        def pv_drain(c):
            q0 = c * 512
            po1, po2, po3 = po_cur["po1"], po_cur["po2"], po_cur["po3"]
            # mid-kernel drains ride the gpsimd ring (sync carries the
            # h1-projection swaps then); the tail drain takes sync
            eng = nc.sync if c == 3 else nc.gpsimd
            ob1 = opool.tile([P, 512], F32, tag="ob1", name=f"ob1_{c}")
            nc.vector.tensor_copy(ob1[:], po1[:])
            eng.dma_start(outV[0:P, q0:q0 + 512], ob1[:])
            ob2 = opool.tile([P, 512], F32, tag="ob2", name=f"ob2_{c}")
            nc.vector.tensor_copy(ob2[:], po2[:])
            eng.dma_start(outV[P:2 * P, q0:q0 + 512], ob2[:])
            obd = opool.tile([P, 512], F32, tag="obd", name=f"obd_{c}")
            nc.vector.tensor_copy(obd[:], po3[:])
            eng.dma_start(
                outD[:, q0:q0 + 512],
                obd.rearrange("(n r) c -> n r c", r=32)[:, 0, :],
            )

        # ---- emission schedule. Program order must respect dataflow (the
        # dep tracker is a linear scan), so producers are interleaved at
        # their proper spots; the score/exp/PV braid is priority-boosted
        # so the scheduler never lets filler (projections, V-projs, late
        # loads) preempt the exp feed — filler runs in PE/DVE idle slots.
        # Emission = intended execution order; projections and V-projs are
        # interleaved into specific m0 phases sized so each phase's PE
        # work fits under that phase's ACT (exp) budget.
        # Skewed pipeline: the m0 pair's scores run one full chunk ahead
        # of the m1 pair's, so ACT always has m0 exps to chew while m1's
        # inputs (projections, V-projs) resolve; chunk c's PV triples
        # braid into the m1 phase of chunk c (ahead of sq(1, c+1, j)'s
        # reuse of the m1 pt tag j).
        nc.sync.dma_start(utri_s[:], utri2)
        proj_rope_h(0, "q", qth[0], 0, nc.sync)
        proj_rope_h(0, "k", kth[0], 0, nc.sync)
        for j in range(4):
            sq(0, 0, j)                      # A: chunk-0 m0
        proj_rope_h(1, "q", qth[1], 0, nc.gpsimd)
        proj_rope_h(1, "k", kth[1], 0, nc.gpsimd)
        for j in range(8):
            sq(0, 1, j)                      # B: chunk-1 m0 (m0-h0 only)
        for j in range(4):
            sq(1, 0, j)                      # C: chunk-0 m1
        load_xb(0)
        load_xb(1)
        nc.sync.dma_start(w_v[:], wvT)
        vproj(0)
        vproj(1)
        proj_rope_h(0, "q", qth[0], 1, nc.sync)
        proj_rope_h(0, "k", kth[0], 1, nc.sync)
        pv_open(0)                           # D: chunk-1 m1 + PV(0)
        for j in range(8):
            if j < 4:
                pv_triple(0, j, j == 3)
            sq(1, 1, j)
        pv_drain(0)
        vproj(2)
        vproj(3)
        proj_rope_h(1, "q", qth[1], 1, nc.gpsimd)
        proj_rope_h(1, "k", kth[1], 1, nc.gpsimd)
        for j in range(12):
            sq(0, 2, j)                      # E: chunk-2 m0
        pv_open(1)                           # F: chunk-2 m1 + PV(1)
        for j in range(12):
            if j < 8:
                pv_triple(1, j, j == 7)
            sq(1, 2, j)
        pv_drain(1)
        vproj(4)
        vproj(5)
        vproj(6)
        vproj(7)
        pv_open(2)                           # G: chunk-3 m0 + PV(2)
        for j in range(16):
            sq(0, 3, j)
            if j < 12:
                pv_triple(2, j, j == 11)
        pv_drain(2)
        pv_open(3)                           # H: chunk-3 m1 + PV(3)
        for j in range(16):
            sq(1, 3, j)
            if j >= 2:
                pv_triple(3, j - 2, False)
        pv_triple(3, 14, False)
        pv_triple(3, 15, True)
        pv_drain(3)

    nc.compile()
    return nc


_CACHE = {}


def _get_nc():
    if "nc" not in _CACHE:
        _CACHE["nc"] = _build_nc()
    return _CACHE["nc"]


def _host_inputs(x, Wq, Wk, Wv):
    bf = ml_dtypes.bfloat16
    f8 = ml_dtypes.float8_e4m3
    # RoPE tables (match reference: theta over hs/2 freqs with dim=n_emb)
    i = np.arange(HS // 2, dtype=np.float32)
    theta = np.float32(10000.0) ** (-2.0 * i / np.float32(CIN))
    pos = np.arange(T, dtype=np.float32)
    ang = pos[:, None] * theta[None, :]
    cosT = np.cos(ang).T.astype(np.float32)  # [32, T]
    sinT = np.sin(ang).T.astype(np.float32)
    cos4 = np.ascontiguousarray(np.tile(cosT, (4, 1))).astype(bf)
    sin4 = np.ascontiguousarray(
        np.tile(np.concatenate([-sinT, sinT], axis=0), (2, 1))
    ).astype(bf)  # rows: [-sin, +sin] x2
    utri_np = np.triu(np.ones((P, P), np.float32))
    utri2_np = np.ascontiguousarray(np.tile(utri_np, (1, 2))).astype(bf)

    perm = np.concatenate([np.arange(0, HS, 2), np.arange(1, HS, 2)])

    def warr(wT):
        # [1024, 256] -> [128, 8*256] so the device DMA is a flat copy
        return np.ascontiguousarray(
            wT.reshape(NCT, P, DOUT).transpose(1, 0, 2).reshape(P, NCT * DOUT)
        )

    in_maps = []
    for core in range(8):
        b, g = core // 4, core % 4
        idx = np.concatenate([(4 * g + h) * HS + perm for h in range(NHC)])
        xT = np.ascontiguousarray(x[b].T)
        m = {
            "xbT": xT.astype(bf),
            "x8T": xT.astype(f8),
            "wq8": warr(Wq[idx].T).astype(f8),
            "wk8": warr(Wk[idx].T).astype(f8),
            "wvT": warr(Wv[g * DOUT:(g + 1) * DOUT].T).astype(bf),
            "cos4": cos4,
            "sin4": sin4,
            "utri2": utri2_np,
        }
        in_maps.append(m)
    return in_maps


def kernel(x, Wq, Wk, Wv, _trace=False, _trace_kwargs=None):
    x = np.asarray(x)
    Wq, Wk, Wv = np.asarray(Wq), np.asarray(Wk), np.asarray(Wv)
    B = x.shape[0]
    nc = _get_nc()
    in_maps = _host_inputs(x, Wq, Wk, Wv)
    res = run_bass_kernel_spmd(
        nc, in_maps, list(range(8)), trace=_trace, **(_trace_kwargs or {})
    )
    out = np.zeros((B, T, CIN), np.float32)
    for core in range(8):
        b, g = core // 4, core % 4
        v = res.results[core]["outV"].reshape(NHC, HS, T)
        d = res.results[core]["outD"].reshape(NHC, 1, T)
        o = v / d
        out[b, :, g * DOUT:(g + 1) * DOUT] = o.reshape(DOUT, T).T
    if _trace:
        return out, res
    return out
